# revision 8
# baseline (speedup 1.0000x reference)
"""CGCNNConv forward on 8 Trainium2 NeuronCores (Bass/Tile, SPMD).

Strategy (edge-sharded by destination-node tiles):
  Host: nodes are bin-packed into 128-node "tiles" balanced by in-degree
  (50 tiles/core x 8 cores). Edges are grouped by the tile of their
  destination (row) node, padded to Gt groups of 128 edges per tile, and
  the three per-edge feature streams (nf[row], nf[col], edge_attr) are
  laid out feature-on-partition so the device streams them sequentially.
  Device (per core, identical program, different data):
    pre1T[hid,e] = W1a^T nf[row]^T + W1b^T nf[col]^T + W1c^T attr^T   (PE)
    h1T = softplus(pre1T + b1)          (ACT: exp then ln(1+x))
    m[e,f] = h1 @ W2                    (PE)
    aggT[f,j] += m^T S_g  (+ b2 (x) deg)  per node tile, S_g one-hot from
      rowlocal via DVE is_equal — this is the segment-sum as matmul.
    node update: pre2T = U1a^T nfT + U1b^T aggT; h2T = softplus(+ub1);
    updT = U2^T h2T; per-feature partial sum/sumsq for BatchNorm.
  Host: combines per-core BN partials (minus padding-slot contributions),
  normalizes, and scatters columns back to original node order.
  ub2 is dropped: BatchNorm is invariant to a per-feature constant shift.
"""

import numpy as np
import ml_dtypes

import concourse.bass as bass
import concourse.bacc as bacc
import concourse.tile as tile
from concourse import mybir
from concourse.bass_utils import run_bass_kernel_spmd

BF16 = ml_dtypes.bfloat16

N, E = 50000, 600000
HD = 128
NCORES = 8
T_PER_CORE = 50
NTILES = NCORES * T_PER_CORE          # 400 tiles of <=128 nodes
P = 128
EPS = 1e-5

LAST_RESULTS = None  # BassKernelResults of the most recent run (for test harness)
TRACE = False


# ----------------------------------------------------------------------------
# Host-side packing
# ----------------------------------------------------------------------------

def _pack(row):
    """Assign nodes to NTILES tiles (<=128 nodes each, in-degree balanced),
    then order edges tile-contiguously and pad each tile to Gt*128 edges.

    Returns dict with packing arrays.
    """
    import heapq

    deg = np.bincount(row, minlength=N).astype(np.int64)
    order = np.argsort(-deg, kind="stable")

    tile_load = np.zeros(NTILES, dtype=np.int64)
    tile_nnodes = np.zeros(NTILES, dtype=np.int32)
    tile_of_node = np.empty(N, dtype=np.int32)
    slot_of_node = np.empty(N, dtype=np.int32)

    heap = [(0, t) for t in range(NTILES)]
    heapq.heapify(heap)
    for v in order:
        while True:
            load, t = heapq.heappop(heap)
            if tile_nnodes[t] < P:
                break
        tile_of_node[v] = t
        slot_of_node[v] = tile_nnodes[t]
        tile_nnodes[t] += 1
        tile_load[t] = load + deg[v]
        if tile_nnodes[t] < P:
            heapq.heappush(heap, (tile_load[t], t))

    max_load = int(tile_load.max())
    gt = max(4, -(-max_load // 512) * 4)  # groups per tile, multiple of 4

    # order tiles: balance load across cores (snake), then core-major
    tile_order = np.argsort(-tile_load, kind="stable")
    core_of_rank = np.empty(NTILES, dtype=np.int32)
    for r in range(NTILES):
        blk, pos = divmod(r, NCORES)
        core_of_rank[r] = pos if blk % 2 == 0 else NCORES - 1 - pos
    # global tile index in core-major order
    new_index = np.empty(NTILES, dtype=np.int32)
    counters = np.zeros(NCORES, dtype=np.int32)
    for r in range(NTILES):
        t_old = tile_order[r]
        c = core_of_rank[r]
        new_index[t_old] = c * T_PER_CORE + counters[c]
        counters[c] += 1
    assert (counters == T_PER_CORE).all()

    gtile_of_node = new_index[tile_of_node]

    # edges sorted by (global tile of row), then padded per tile
    tile_of_edge = gtile_of_node[row]
    eorder = np.argsort(tile_of_edge, kind="stable")
    counts = np.bincount(tile_of_edge, minlength=NTILES)
    assert counts.max() <= gt * P

    cap = gt * P
    perm = np.full(NTILES * cap, -1, dtype=np.int64)
    starts = np.zeros(NTILES + 1, dtype=np.int64)
    np.cumsum(counts, out=starts[1:])
    idx_within = np.arange(E) - starts[tile_of_edge[eorder]]
    perm[tile_of_edge[eorder] * cap + idx_within] = eorder

    return dict(
        gt=gt,
        perm=perm,                      # [NTILES*cap] edge id or -1
        gtile_of_node=gtile_of_node,
        slot_of_node=slot_of_node,
        tile_nnodes_new=np.bincount(gtile_of_node, minlength=NTILES),
        deg=deg,
    )


def _prep(node_features, edge_index, edge_attr):
    """Build per-core input arrays. Returns (in_maps, meta)."""
    row = np.asarray(edge_index[0])
    col = np.asarray(edge_index[1])
    pk = _pack(row)
    gt = pk["gt"]
    perm = pk["perm"]
    cap = gt * P

    nf = np.asarray(node_features, dtype=np.float32)
    ea = np.asarray(edge_attr, dtype=np.float32)
    nf_bf = nf.astype(BF16)
    ea_bf = ea.astype(BF16)

    sel = np.where(perm >= 0, perm, 0)
    valid = perm >= 0

    rowlocal = np.where(valid, pk["slot_of_node"][row[sel]], -1).astype(np.float32)

    # feature streams, [tile, f, (sgroup, ktile, e)] layout
    n_sg_tile = cap // 512                      # supergroups per tile
    def stream(x):  # x: [NTILES*cap, 128] -> [NTILES, 128, n_sg*3*512] slot blocks
        return x.reshape(NTILES, n_sg_tile, 512, P).transpose(0, 1, 3, 2)

    rf = stream(nf_bf[row[sel]])
    cf = stream(nf_bf[col[sel]])
    af = stream(ea_bf[sel])
    # edge3T[tile, f, sg, k, e] — partition (f) major to match the SBUF tile's
    # AP iteration order, so the load DMA is a straight element stream.
    edge3T = np.stack([rf, cf, af], axis=3)     # [NTILES, n_sg, 128, 3, 512]
    edge3T = np.ascontiguousarray(edge3T.transpose(0, 2, 1, 3, 4)).reshape(
        NTILES, P, n_sg_tile * 3 * 512)

    # rowlocal, [128, G] per core (partition = edge-in-group)
    rowlocal_p = rowlocal.reshape(NTILES * gt, P).T.copy()  # [128, NTILES*gt]

    # per-slot node ids / degrees
    slots_node = np.full(NTILES * P, -1, dtype=np.int64)
    gslot = pk["gtile_of_node"].astype(np.int64) * P + pk["slot_of_node"]
    slots_node[gslot] = np.arange(N)
    deg_slot = np.zeros(NTILES * P, dtype=np.float32)
    deg_slot[gslot] = pk["deg"]

    nfT_slots = np.zeros((NTILES * P, P), dtype=np.float32)
    nfT_slots[gslot] = nf
    nfT_slots = nfT_slots.astype(BF16)

    in_maps = []
    npc = T_PER_CORE * P  # node slots per core
    for c in range(NCORES):
        tsl = slice(c * T_PER_CORE, (c + 1) * T_PER_CORE)
        gsl = slice(c * T_PER_CORE * gt, (c + 1) * T_PER_CORE * gt)
        ssl = slice(c * npc, (c + 1) * npc)
        in_maps.append(dict(
            edge3T=np.ascontiguousarray(edge3T[tsl]),
            rowlocal=np.ascontiguousarray(rowlocal_p[:, gsl]),
            nfT=np.ascontiguousarray(nfT_slots[ssl].T),          # [128, npc] bf16
            deg_r=np.ascontiguousarray(deg_slot[ssl][None, :].astype(BF16)),
        ))

    meta = dict(gt=gt, slots_node=slots_node, n_fake=int((slots_node < 0).sum()))
    return in_maps, meta


# ----------------------------------------------------------------------------
# Device program
# ----------------------------------------------------------------------------

_PROG_CACHE = {}


def _patch_act_tables():
    """Force the act-table-load pass to pick natural_log_exp_and_others
    (which covers Exp, Ln, Copy, Square) for every activation. The default
    pass assigns each function its first containing set, which alternates
    exp_and_others/natural_log and inserts a 1.28us table reload per
    softplus — ~210us of pure ACT stall per core."""
    import concourse.hw_specs as hw_specs
    orig = hw_specs.get_activation_tables

    def only_covering(arch):
        tables = orig(arch)
        keep = "natural_log_exp_and_others"
        return {name: (funcs if name == keep else set())
                for name, funcs in tables.items()}

    only_covering.__wrapped__ = orig
    bacc.get_activation_tables = only_covering


_patch_act_tables()


def _build(gt):
    if gt in _PROG_CACHE:
        return _PROG_CACHE[gt]

    n_sg_tile = gt // 4
    npc = T_PER_CORE * P
    G = T_PER_CORE * gt               # groups per core
    SG = G // 4                        # supergroups per core
    F32 = mybir.dt.float32
    BF = mybir.dt.bfloat16
    AF = mybir.ActivationFunctionType

    nc = bacc.Bacc(target_bir_lowering=False)

    edge3T = nc.dram_tensor("edge3T", [T_PER_CORE, P, n_sg_tile * 3 * 512], BF, kind="ExternalInput")
    rowlocal = nc.dram_tensor("rowlocal", [P, G], F32, kind="ExternalInput")
    nfT = nc.dram_tensor("nfT", [P, npc], BF, kind="ExternalInput")
    deg_r = nc.dram_tensor("deg_r", [1, npc], BF, kind="ExternalInput")
    W123 = nc.dram_tensor("W123", [P, 3, HD], BF, kind="ExternalInput")
    W2 = nc.dram_tensor("W2", [P, HD], BF, kind="ExternalInput")
    U1a = nc.dram_tensor("U1a", [P, HD], BF, kind="ExternalInput")
    U1b = nc.dram_tensor("U1b", [P, HD], BF, kind="ExternalInput")
    U2 = nc.dram_tensor("U2", [P, HD], BF, kind="ExternalInput")
    b2_1 = nc.dram_tensor("b2_1", [1, HD], BF, kind="ExternalInput")
    b1_t = nc.dram_tensor("b1_t", [P, 1], F32, kind="ExternalInput")
    ub1_t = nc.dram_tensor("ub1_t", [P, 1], F32, kind="ExternalInput")
    iota_f = nc.dram_tensor("iota_f", [P, P], F32, kind="ExternalInput")

    updT_out = nc.dram_tensor("updT_out", [P, npc], F32, kind="ExternalOutput")
    stats_out = nc.dram_tensor("stats_out", [P, 2], F32, kind="ExternalOutput")

    with tile.TileContext(nc) as tc:
        with (
            tc.tile_pool(name="single", bufs=1) as single,
            tc.tile_pool(name="stream", bufs=3) as stream,
            tc.tile_pool(name="work", bufs=3) as work,
            tc.tile_pool(name="ps_pre", bufs=2, space="PSUM") as ps_pre,
            tc.tile_pool(name="ps_m", bufs=2, space="PSUM") as ps_m,
            tc.tile_pool(name="ps_agg", bufs=2, space="PSUM") as ps_agg,
        ):
            # ---- constants ----
            w123_sb = single.tile([P, 3, HD], BF)
            nc.sync.dma_start(out=w123_sb, in_=W123[:])
            w2_sb = single.tile([P, HD], BF)
            nc.sync.dma_start(out=w2_sb, in_=W2[:])
            u1a_sb = single.tile([P, HD], BF)
            nc.sync.dma_start(out=u1a_sb, in_=U1a[:])
            u1b_sb = single.tile([P, HD], BF)
            nc.sync.dma_start(out=u1b_sb, in_=U1b[:])
            u2_sb = single.tile([P, HD], BF)
            nc.sync.dma_start(out=u2_sb, in_=U2[:])
            b2_sb = single.tile([1, HD], BF)
            nc.sync.dma_start(out=b2_sb, in_=b2_1[:])
            b1_sb = single.tile([P, 1], F32)
            nc.sync.dma_start(out=b1_sb, in_=b1_t[:])
            ub1_sb = single.tile([P, 1], F32)
            nc.sync.dma_start(out=ub1_sb, in_=ub1_t[:])
            iota_sb = single.tile([P, P], F32)
            nc.sync.dma_start(out=iota_sb, in_=iota_f[:])
            rowloc_sb = single.tile([P, G], F32)
            nc.sync.dma_start(out=rowloc_sb, in_=rowlocal[:])
            nfT_sb = single.tile([P, npc], BF)
            nc.sync.dma_start(out=nfT_sb, in_=nfT[:])
            deg_sb = single.tile([1, npc], BF)
            nc.sync.dma_start(out=deg_sb, in_=deg_r[:])

            aggT_sb = single.tile([P, npc], BF)
            stats_sb = single.tile([P, 2, 16], F32)
            nc.vector.memset(stats_sb, 0.0)

            # ---- edge phase ----
            for t in range(T_PER_CORE):
                e3_sb = stream.tile([P, n_sg_tile, 3, 512], BF, tag="e3")
                nc.sync.dma_start(out=e3_sb, in_=edge3T[t])

                agg_ps = ps_agg.tile([P, P], F32, space="PSUM", tag="agg")
                # b2 * deg outer product opens the accumulation group
                nc.tensor.matmul(
                    out=agg_ps[:], lhsT=b2_sb[:],
                    rhs=deg_sb[:, t * P:(t + 1) * P],
                    start=True, stop=False,
                )
                for si in range(n_sg_tile):
                    s = t * n_sg_tile + si
                    pre_ps = ps_pre.tile([P, 512], F32, space="PSUM", tag="pre")
                    for k in range(3):
                        nc.tensor.matmul(
                            out=pre_ps[:],
                            lhsT=w123_sb[:, k, :],
                            rhs=e3_sb[:, si, k, :],
                            start=(k == 0), stop=(k == 2),
                        )
                    # softplus = ln(1 + exp(x + b1))
                    sp_sb = work.tile([P, 512], F32, tag="sp")
                    nc.scalar.activation(out=sp_sb[:], in_=pre_ps[:], func=AF.Exp,
                                         bias=b1_sb[:], scale=1.0)
                    h1_sb = work.tile([P, 512], BF, tag="h1")
                    nc.scalar.activation(out=h1_sb[:], in_=sp_sb[:], func=AF.Ln,
                                         bias=1.0, scale=1.0)

                    m_ps = ps_m.tile([P, 4, P], F32, space="PSUM", tag="m")
                    for i in range(4):
                        nc.tensor.matmul(
                            out=m_ps[:, i, :],
                            lhsT=h1_sb[:, i * P:(i + 1) * P],
                            rhs=w2_sb[:],
                            start=True, stop=True,
                        )
                    m_sb = work.tile([P, 4, P], BF, tag="m_sb")
                    nc.vector.tensor_copy(out=m_sb[:], in_=m_ps[:])

                    for i in range(4):
                        g = s * 4 + i
                        s_sb = work.tile([P, P], BF, tag="s_one")
                        nc.gpsimd.tensor_scalar(
                            out=s_sb[:], in0=iota_sb[:],
                            scalar1=rowloc_sb[:, g:g + 1], scalar2=None,
                            op0=mybir.AluOpType.is_equal,
                        )
                        nc.tensor.matmul(
                            out=agg_ps[:], lhsT=m_sb[:, i, :], rhs=s_sb[:],
                            start=False, stop=(si == n_sg_tile - 1 and i == 3),
                        )
                nc.scalar.copy(out=aggT_sb[:, t * P:(t + 1) * P], in_=agg_ps[:])

            # ---- node phase (4 tiles = 512 nodes per block) ----
            nblk = T_PER_CORE // 4  # 12 full blocks + remainder below
            rem = T_PER_CORE % 4
            for blk in range(nblk + (1 if rem else 0)):
                j0 = blk * 512
                w = 512 if blk < nblk else rem * P
                pre2_ps = ps_pre.tile([P, 512], F32, space="PSUM", tag="pre")
                nc.tensor.matmul(out=pre2_ps[:, :w], lhsT=u1a_sb[:],
                                 rhs=nfT_sb[:, j0:j0 + w], start=True, stop=False)
                nc.tensor.matmul(out=pre2_ps[:, :w], lhsT=u1b_sb[:],
                                 rhs=aggT_sb[:, j0:j0 + w], start=False, stop=True)
                sp2_sb = work.tile([P, 512], F32, tag="sp")
                nc.scalar.activation(out=sp2_sb[:, :w], in_=pre2_ps[:, :w],
                                     func=AF.Exp, bias=ub1_sb[:], scale=1.0)
                h2_sb = work.tile([P, 512], BF, tag="h1")
                nc.scalar.activation(out=h2_sb[:, :w], in_=sp2_sb[:, :w],
                                     func=AF.Ln, bias=1.0, scale=1.0)
                upd_ps = ps_m.tile([P, 512], F32, space="PSUM", tag="m")
                nc.tensor.matmul(out=upd_ps[:, :w], lhsT=u2_sb[:],
                                 rhs=h2_sb[:, :w], start=True, stop=True)
                upd_sb = work.tile([P, 512], F32, tag="upd")
                nc.vector.tensor_copy(out=upd_sb[:, :w], in_=upd_ps[:, :w])
                nc.sync.dma_start(out=updT_out[:, j0:j0 + w], in_=upd_sb[:, :w])
                # stats
                sq_sb = work.tile([P, 512], F32, tag="sq")
                nc.vector.tensor_mul(out=sq_sb[:, :w], in0=upd_sb[:, :w], in1=upd_sb[:, :w])
                nc.vector.tensor_reduce(out=stats_sb[:, 0, blk:blk + 1], in_=upd_sb[:, :w],
                                        axis=mybir.AxisListType.X, op=mybir.AluOpType.add)
                nc.vector.tensor_reduce(out=stats_sb[:, 1, blk:blk + 1], in_=sq_sb[:, :w],
                                        axis=mybir.AxisListType.X, op=mybir.AluOpType.add)

            stats2_sb = single.tile([P, 2], F32)
            nc.vector.tensor_reduce(out=stats2_sb[:], in_=stats_sb[:],
                                    axis=mybir.AxisListType.X, op=mybir.AluOpType.add)
            nc.sync.dma_start(out=stats_out[:], in_=stats2_sb[:])

    nc.compile()
    _PROG_CACHE[gt] = nc
    return nc


# ----------------------------------------------------------------------------
# Entry point
# ----------------------------------------------------------------------------

def prepare(node_features, edge_index, edge_attr, W1, b1, W2, b2,
            U1, ub1, U2, ub2, gamma, beta):
    """Host prep + program build. Returns (nc, in_maps, meta)."""
    node_features = np.asarray(node_features, dtype=np.float32)
    edge_index = np.asarray(edge_index)
    edge_attr = np.asarray(edge_attr, dtype=np.float32)
    W1 = np.asarray(W1, dtype=np.float32); b1 = np.asarray(b1, dtype=np.float32)
    W2 = np.asarray(W2, dtype=np.float32); b2 = np.asarray(b2, dtype=np.float32)
    U1 = np.asarray(U1, dtype=np.float32); ub1 = np.asarray(ub1, dtype=np.float32)
    U2 = np.asarray(U2, dtype=np.float32)

    in_maps, meta = _prep(node_features, edge_index, edge_attr)
    gt = meta["gt"]

    w123 = np.stack([W1[:P], W1[P:2 * P], W1[2 * P:]], axis=1).astype(BF16)  # [128,3,128]
    shared = dict(
        W123=w123,
        W2=W2.astype(BF16),
        U1a=U1[:P].astype(BF16),
        U1b=U1[P:].astype(BF16),
        U2=U2.astype(BF16),
        b2_1=b2[None, :].astype(BF16),
        b1_t=b1[:, None].astype(np.float32),
        ub1_t=ub1[:, None].astype(np.float32),
        iota_f=np.broadcast_to(np.arange(P, dtype=np.float32), (P, P)).copy(),
    )
    for m in in_maps:
        m.update(shared)
    nc = _build(gt)
    return nc, in_maps, meta


def finalize(res_results, meta, U1, ub1, U2, gamma, beta):
    """BatchNorm finalize + unshard on host."""
    slots_node = meta["slots_node"]
    updT = np.concatenate([r["updT_out"] for r in res_results], axis=1)
    stats = np.stack([r["stats_out"] for r in res_results]).astype(np.float64)

    ub1 = np.asarray(ub1, dtype=np.float64)
    U2_64 = np.asarray(U2, dtype=np.float64)
    gamma = np.asarray(gamma, dtype=np.float64)
    beta = np.asarray(beta, dtype=np.float64)

    # remove padding-slot contributions (identical constant column each)
    c_fake = np.log1p(np.exp(ub1)) @ U2_64
    n_fake = meta["n_fake"]
    tot_sum = stats[:, :, 0].sum(axis=0) - n_fake * c_fake
    tot_sq = stats[:, :, 1].sum(axis=0) - n_fake * c_fake ** 2
    mu = tot_sum / N
    var = tot_sq / N - mu ** 2
    scale = gamma / np.sqrt(var + EPS)
    shift = beta - mu * scale

    keep = slots_node >= 0
    out = np.empty((N, P), dtype=np.float32)
    out[slots_node[keep]] = (updT[:, keep].astype(np.float64) * scale[:, None]
                             + shift[:, None]).T.astype(np.float32)
    return out


def kernel(node_features, edge_index, edge_attr, W1, b1, W2, b2,
           U1, ub1, U2, ub2, gamma, beta):
    global LAST_RESULTS
    nc, in_maps, meta = prepare(node_features, edge_index, edge_attr, W1, b1,
                                W2, b2, U1, ub1, U2, ub2, gamma, beta)
    res = run_bass_kernel_spmd(nc, in_maps, core_ids=list(range(NCORES)), trace=TRACE)
    LAST_RESULTS = res
    return finalize(res.results, meta, U1, ub1, U2, gamma, beta)




# revision 13
# speedup vs baseline: 4.6213x; 4.6213x over previous
"""CGCNNConv forward on 8 Trainium2 NeuronCores (Bass/Tile, SPMD).

Strategy (edge-sharded by destination-node tiles):
  Host: nodes are bin-packed into 128-node "tiles" balanced by in-degree
  (50 tiles/core x 8 cores). Edges are grouped by the tile of their
  destination (row) node, padded to Gt groups of 128 edges per tile, and
  the three per-edge feature streams (nf[row], nf[col], edge_attr) are
  laid out feature-on-partition so the device streams them sequentially.
  Device (per core, identical program, different data):
    pre1T[hid,e] = W1a^T nf[row]^T + W1b^T nf[col]^T + W1c^T attr^T   (PE)
    h1T = softplus(pre1T + b1)          (ACT: exp then ln(1+x))
    m[e,f] = h1 @ W2                    (PE)
    aggT[f,j] += m^T S_g  (+ b2 (x) deg)  per node tile, S_g one-hot from
      rowlocal via DVE is_equal — this is the segment-sum as matmul.
    node update: pre2T = U1a^T nfT + U1b^T aggT; h2T = softplus(+ub1);
    updT = U2^T h2T; per-feature partial sum/sumsq for BatchNorm.
  Host: combines per-core BN partials (minus padding-slot contributions),
  normalizes, and scatters columns back to original node order.
  ub2 is dropped: BatchNorm is invariant to a per-feature constant shift.
"""

import numpy as np
import ml_dtypes

import concourse.bass as bass
import concourse.bacc as bacc
import concourse.tile as tile
from concourse import mybir
from concourse.bass_utils import run_bass_kernel_spmd

BF16 = ml_dtypes.bfloat16

N, E = 50000, 600000
HD = 128
NCORES = 8
T_PER_CORE = 50
NTILES = NCORES * T_PER_CORE          # 400 tiles of <=128 nodes
P = 128
EPS = 1e-5

LAST_RESULTS = None  # BassKernelResults of the most recent run (for test harness)
TRACE = False


# ----------------------------------------------------------------------------
# Host-side packing
# ----------------------------------------------------------------------------

def _pack(row):
    """Assign nodes to NTILES tiles (<=128 nodes each, in-degree balanced),
    then order edges tile-contiguously and pad each tile to Gt*128 edges.

    Returns dict with packing arrays.
    """
    import heapq

    deg = np.bincount(row, minlength=N).astype(np.int64)
    order = np.argsort(-deg, kind="stable")

    tile_load = np.zeros(NTILES, dtype=np.int64)
    tile_nnodes = np.zeros(NTILES, dtype=np.int32)
    tile_of_node = np.empty(N, dtype=np.int32)
    slot_of_node = np.empty(N, dtype=np.int32)

    heap = [(0, t) for t in range(NTILES)]
    heapq.heapify(heap)
    for v in order:
        while True:
            load, t = heapq.heappop(heap)
            if tile_nnodes[t] < P:
                break
        tile_of_node[v] = t
        slot_of_node[v] = tile_nnodes[t]
        tile_nnodes[t] += 1
        tile_load[t] = load + deg[v]
        if tile_nnodes[t] < P:
            heapq.heappush(heap, (tile_load[t], t))

    max_load = int(tile_load.max())
    gt = max(4, -(-max_load // 512) * 4)  # groups per tile, multiple of 4

    # order tiles: balance load across cores (snake), then core-major
    tile_order = np.argsort(-tile_load, kind="stable")
    core_of_rank = np.empty(NTILES, dtype=np.int32)
    for r in range(NTILES):
        blk, pos = divmod(r, NCORES)
        core_of_rank[r] = pos if blk % 2 == 0 else NCORES - 1 - pos
    # global tile index in core-major order
    new_index = np.empty(NTILES, dtype=np.int32)
    counters = np.zeros(NCORES, dtype=np.int32)
    for r in range(NTILES):
        t_old = tile_order[r]
        c = core_of_rank[r]
        new_index[t_old] = c * T_PER_CORE + counters[c]
        counters[c] += 1
    assert (counters == T_PER_CORE).all()

    gtile_of_node = new_index[tile_of_node]

    # edges sorted by (global tile of row), then padded per tile
    tile_of_edge = gtile_of_node[row]
    eorder = np.argsort(tile_of_edge, kind="stable")
    counts = np.bincount(tile_of_edge, minlength=NTILES)
    assert counts.max() <= gt * P

    cap = gt * P
    perm = np.full(NTILES * cap, -1, dtype=np.int64)
    starts = np.zeros(NTILES + 1, dtype=np.int64)
    np.cumsum(counts, out=starts[1:])
    idx_within = np.arange(E) - starts[tile_of_edge[eorder]]
    perm[tile_of_edge[eorder] * cap + idx_within] = eorder

    return dict(
        gt=gt,
        perm=perm,                      # [NTILES*cap] edge id or -1
        gtile_of_node=gtile_of_node,
        slot_of_node=slot_of_node,
        tile_nnodes_new=np.bincount(gtile_of_node, minlength=NTILES),
        deg=deg,
    )


def _prep(node_features, edge_index, edge_attr):
    """Build per-core input arrays. Returns (in_maps, meta)."""
    row = np.asarray(edge_index[0])
    col = np.asarray(edge_index[1])
    pk = _pack(row)
    gt = pk["gt"]
    perm = pk["perm"]
    cap = gt * P

    nf = np.asarray(node_features, dtype=np.float32)
    ea = np.asarray(edge_attr, dtype=np.float32)
    nf_bf = nf.astype(BF16)
    ea_bf = ea.astype(BF16)

    sel = np.where(perm >= 0, perm, 0)
    valid = perm >= 0

    rowlocal = np.where(valid, pk["slot_of_node"][row[sel]], -1).astype(np.float32)

    # feature streams, [tile, f, (sgroup, ktile, e)] layout
    n_sg_tile = cap // 512                      # supergroups per tile
    def stream(x):  # x: [NTILES*cap, 128] -> [NTILES, 128, n_sg*3*512] slot blocks
        return x.reshape(NTILES, n_sg_tile, 512, P).transpose(0, 1, 3, 2)

    rf = stream(nf_bf[row[sel]])
    cf = stream(nf_bf[col[sel]])
    af = stream(ea_bf[sel])
    # edge3T[tile, f, sg, k, e] — partition (f) major to match the SBUF tile's
    # AP iteration order, so the load DMA is a straight element stream.
    edge3T = np.stack([rf, cf, af], axis=3)     # [NTILES, n_sg, 128, 3, 512]
    edge3T = np.ascontiguousarray(edge3T.transpose(0, 2, 1, 3, 4)).reshape(
        NTILES, P, n_sg_tile * 3 * 512)

    # rowlocal, [128, G] per core (partition = edge-in-group)
    rowlocal_p = rowlocal.reshape(NTILES * gt, P).T.copy()  # [128, NTILES*gt]

    # per-slot node ids / degrees
    slots_node = np.full(NTILES * P, -1, dtype=np.int64)
    gslot = pk["gtile_of_node"].astype(np.int64) * P + pk["slot_of_node"]
    slots_node[gslot] = np.arange(N)
    deg_slot = np.zeros(NTILES * P, dtype=np.float32)
    deg_slot[gslot] = pk["deg"]

    nfT_slots = np.zeros((NTILES * P, P), dtype=np.float32)
    nfT_slots[gslot] = nf
    nfT_slots = nfT_slots.astype(BF16)

    in_maps = []
    npc = T_PER_CORE * P  # node slots per core
    for c in range(NCORES):
        tsl = slice(c * T_PER_CORE, (c + 1) * T_PER_CORE)
        gsl = slice(c * T_PER_CORE * gt, (c + 1) * T_PER_CORE * gt)
        ssl = slice(c * npc, (c + 1) * npc)
        in_maps.append(dict(
            edge3T=np.ascontiguousarray(edge3T[tsl]),
            rowlocal=np.ascontiguousarray(rowlocal_p[:, gsl]),
            nfT=np.ascontiguousarray(nfT_slots[ssl].T),          # [128, npc] bf16
            deg_r=np.ascontiguousarray(deg_slot[ssl][None, :].astype(BF16)),
        ))

    meta = dict(gt=gt, slots_node=slots_node, n_fake=int((slots_node < 0).sum()))
    return in_maps, meta


# ----------------------------------------------------------------------------
# Device program
# ----------------------------------------------------------------------------

_PROG_CACHE = {}


def _patch_act_tables():
    """Force the act-table-load pass to pick natural_log_exp_and_others
    (which covers Exp, Ln, Copy, Square) for every activation. The default
    pass assigns each function its first containing set, which alternates
    exp_and_others/natural_log and inserts a 1.28us table reload per
    softplus — ~210us of pure ACT stall per core."""
    import concourse.hw_specs as hw_specs
    orig = hw_specs.get_activation_tables

    def only_covering(arch):
        tables = orig(arch)
        keep = "natural_log_exp_and_others"
        return {name: (funcs if name == keep else set())
                for name, funcs in tables.items()}

    only_covering.__wrapped__ = orig
    bacc.get_activation_tables = only_covering


_patch_act_tables()


def _build(gt, HAS_B1, HAS_B2):
    key = (gt, HAS_B1, HAS_B2)
    if key in _PROG_CACHE:
        return _PROG_CACHE[key]

    n_sg_tile = gt // 4
    npc = T_PER_CORE * P
    G = T_PER_CORE * gt               # groups per core
    F32 = mybir.dt.float32
    BF = mybir.dt.bfloat16
    AF = mybir.ActivationFunctionType

    nc = bacc.Bacc(target_bir_lowering=False)

    edge3T = nc.dram_tensor("edge3T", [T_PER_CORE, P, n_sg_tile * 3 * 512], BF, kind="ExternalInput")
    rowlocal = nc.dram_tensor("rowlocal", [P, G], F32, kind="ExternalInput")
    nfT = nc.dram_tensor("nfT", [P, npc], BF, kind="ExternalInput")
    W123 = nc.dram_tensor("W123", [P, 3, HD], BF, kind="ExternalInput")
    W2 = nc.dram_tensor("W2", [P, HD], BF, kind="ExternalInput")
    U1a = nc.dram_tensor("U1a", [P, HD], BF, kind="ExternalInput")
    U1b = nc.dram_tensor("U1b", [P, HD], BF, kind="ExternalInput")
    U2 = nc.dram_tensor("U2", [P, HD], BF, kind="ExternalInput")
    ub1_t = nc.dram_tensor("ub1_t", [P, 1], F32, kind="ExternalInput")
    iota_f = nc.dram_tensor("iota_f", [P, P], F32, kind="ExternalInput")
    if HAS_B1:
        ones_r = nc.dram_tensor("ones_r", [1, P], BF, kind="ExternalInput")
        b1_r = nc.dram_tensor("b1_r", [1, HD], BF, kind="ExternalInput")
    if HAS_B2:
        b2_1 = nc.dram_tensor("b2_1", [1, HD], BF, kind="ExternalInput")
        deg_r = nc.dram_tensor("deg_r", [1, npc], BF, kind="ExternalInput")

    updT_out = nc.dram_tensor("updT_out", [P, npc], F32, kind="ExternalOutput")
    stats_out = nc.dram_tensor("stats_out", [P, 2], F32, kind="ExternalOutput")

    with tile.TileContext(nc) as tc:
        with (
            tc.tile_pool(name="single", bufs=1) as single,
            tc.tile_pool(name="stream", bufs=3) as stream,
            tc.tile_pool(name="work", bufs=3) as work,
            tc.tile_pool(name="ps_pre", bufs=2, space="PSUM") as ps_pre,
            tc.tile_pool(name="ps_m", bufs=2, space="PSUM") as ps_m,
            tc.tile_pool(name="ps_agg", bufs=2, space="PSUM") as ps_agg,
        ):
            # ---- constants ----
            w123_sb = single.tile([P, 3, HD], BF)
            nc.sync.dma_start(out=w123_sb, in_=W123[:])
            w2_sb = single.tile([P, HD], BF)
            nc.sync.dma_start(out=w2_sb, in_=W2[:])
            u1a_sb = single.tile([P, HD], BF)
            nc.sync.dma_start(out=u1a_sb, in_=U1a[:])
            u1b_sb = single.tile([P, HD], BF)
            nc.sync.dma_start(out=u1b_sb, in_=U1b[:])
            u2_sb = single.tile([P, HD], BF)
            nc.sync.dma_start(out=u2_sb, in_=U2[:])
            ub1_sb = single.tile([P, 1], F32)
            nc.sync.dma_start(out=ub1_sb, in_=ub1_t[:])
            iota_sb = single.tile([P, P], F32)
            nc.sync.dma_start(out=iota_sb, in_=iota_f[:])
            rowloc_sb = single.tile([P, G], F32)
            nc.sync.dma_start(out=rowloc_sb, in_=rowlocal[:])
            nfT_sb = single.tile([P, npc], BF)
            nc.sync.dma_start(out=nfT_sb, in_=nfT[:])
            if HAS_B1:
                ones_sb = single.tile([1, P], BF)
                nc.sync.dma_start(out=ones_sb, in_=ones_r[:])
                b1r_sb = single.tile([1, HD], BF)
                nc.sync.dma_start(out=b1r_sb, in_=b1_r[:])
            if HAS_B2:
                b2_sb = single.tile([1, HD], BF)
                nc.sync.dma_start(out=b2_sb, in_=b2_1[:])
                deg_sb = single.tile([1, npc], BF)
                nc.sync.dma_start(out=deg_sb, in_=deg_r[:])

            aggp_sb = single.tile([P, npc], BF)   # pre-W2 aggregation [h, slot]
            stats_sb = single.tile([P, 2, 16], F32)
            nc.vector.memset(stats_sb, 0.0)

            # ---- edge phase (edge-on-partition) ----
            # pre1[e,h] = sum_k stream_k^T W1k  (streams are lhsT);
            # h1 = softplus(pre1); agg'[h,j] += h1_g^T S_g  per tile.
            # W2 is folded after aggregation (12x fewer elements).
            for t in range(T_PER_CORE):
                e3_sb = stream.tile([P, n_sg_tile, 3, 512], BF, tag="e3")
                nc.sync.dma_start(out=e3_sb, in_=edge3T[t])

                agg_ps = ps_agg.tile([P, P], F32, space="PSUM", tag="agg")
                first_mm = [True]
                for si in range(n_sg_tile):
                    s = t * n_sg_tile + si
                    pre_ps = ps_pre.tile([P, 4, P], F32, space="PSUM", tag="pre")
                    for i in range(4):
                        for k in range(3):
                            nc.tensor.matmul(
                                out=pre_ps[:, i, :],
                                lhsT=e3_sb[:, si, k, i * P:(i + 1) * P],
                                rhs=w123_sb[:, k, :],
                                start=(k == 0), stop=(k == 2 and not HAS_B1),
                            )
                        if HAS_B1:
                            nc.tensor.matmul(
                                out=pre_ps[:, i, :], lhsT=ones_sb[:],
                                rhs=b1r_sb[:], start=False, stop=True)
                    # softplus = ln(1 + exp(x)); b1 handled via K=1 matmul above
                    sp_sb = work.tile([P, 512], F32, tag="sp")
                    nc.scalar.activation(out=sp_sb[:], in_=pre_ps[:], func=AF.Exp,
                                         bias=0.0, scale=1.0)
                    h1_sb = work.tile([P, 4, P], BF, tag="h1")
                    nc.scalar.activation(out=h1_sb[:], in_=sp_sb[:], func=AF.Ln,
                                         bias=1.0, scale=1.0)

                    for i in range(4):
                        g = s * 4 + i
                        s_sb = work.tile([P, P], BF, tag="s_one")
                        nc.vector.tensor_scalar(
                            out=s_sb[:], in0=iota_sb[:],
                            scalar1=rowloc_sb[:, g:g + 1], scalar2=None,
                            op0=mybir.AluOpType.is_equal,
                        )
                        nc.tensor.matmul(
                            out=agg_ps[:], lhsT=h1_sb[:, i, :], rhs=s_sb[:],
                            start=first_mm[0],
                            stop=(si == n_sg_tile - 1 and i == 3),
                        )
                        first_mm[0] = False
                nc.scalar.copy(out=aggp_sb[:, t * P:(t + 1) * P], in_=agg_ps[:])

            # ---- node phase (4 tiles = 512 nodes per block) ----
            nblk = -(-T_PER_CORE // 4)
            for blk in range(nblk):
                j0 = blk * 512
                w = min(512, npc - j0)
                # aggT[f,j] = W2^T agg' (+ b2 (x) deg)
                aggT_ps = ps_m.tile([P, 512], F32, space="PSUM", tag="aggT")
                nc.tensor.matmul(out=aggT_ps[:, :w], lhsT=w2_sb[:],
                                 rhs=aggp_sb[:, j0:j0 + w],
                                 start=True, stop=not HAS_B2)
                if HAS_B2:
                    nc.tensor.matmul(out=aggT_ps[:, :w], lhsT=b2_sb[:],
                                     rhs=deg_sb[:, j0:j0 + w],
                                     start=False, stop=True)
                aggT_bf = work.tile([P, 512], BF, tag="aggT_bf")
                nc.vector.tensor_copy(out=aggT_bf[:, :w], in_=aggT_ps[:, :w])

                pre2_ps = ps_pre.tile([P, 512], F32, space="PSUM", tag="pre")
                nc.tensor.matmul(out=pre2_ps[:, :w], lhsT=u1a_sb[:],
                                 rhs=nfT_sb[:, j0:j0 + w], start=True, stop=False)
                nc.tensor.matmul(out=pre2_ps[:, :w], lhsT=u1b_sb[:],
                                 rhs=aggT_bf[:, :w], start=False, stop=True)
                sp2_sb = work.tile([P, 512], F32, tag="sp")
                nc.scalar.activation(out=sp2_sb[:, :w], in_=pre2_ps[:, :w],
                                     func=AF.Exp, bias=ub1_sb[:], scale=1.0)
                h2_sb = work.tile([P, 512], BF, tag="h1")
                nc.scalar.activation(out=h2_sb[:, :w], in_=sp2_sb[:, :w],
                                     func=AF.Ln, bias=1.0, scale=1.0)
                upd_ps = ps_m.tile([P, 512], F32, space="PSUM", tag="aggT")
                nc.tensor.matmul(out=upd_ps[:, :w], lhsT=u2_sb[:],
                                 rhs=h2_sb[:, :w], start=True, stop=True)
                upd_sb = work.tile([P, 512], F32, tag="upd")
                nc.vector.tensor_copy(out=upd_sb[:, :w], in_=upd_ps[:, :w])
                nc.sync.dma_start(out=updT_out[:, j0:j0 + w], in_=upd_sb[:, :w])
                # stats
                sq_sb = work.tile([P, 512], F32, tag="sq")
                nc.vector.tensor_mul(out=sq_sb[:, :w], in0=upd_sb[:, :w], in1=upd_sb[:, :w])
                nc.vector.tensor_reduce(out=stats_sb[:, 0, blk:blk + 1], in_=upd_sb[:, :w],
                                        axis=mybir.AxisListType.X, op=mybir.AluOpType.add)
                nc.vector.tensor_reduce(out=stats_sb[:, 1, blk:blk + 1], in_=sq_sb[:, :w],
                                        axis=mybir.AxisListType.X, op=mybir.AluOpType.add)

            stats2_sb = single.tile([P, 2], F32)
            nc.vector.tensor_reduce(out=stats2_sb[:], in_=stats_sb[:],
                                    axis=mybir.AxisListType.X, op=mybir.AluOpType.add)
            nc.sync.dma_start(out=stats_out[:], in_=stats2_sb[:])

    nc.compile()
    _PROG_CACHE[key] = nc
    return nc


# ----------------------------------------------------------------------------
# Entry point
# ----------------------------------------------------------------------------

def prepare(node_features, edge_index, edge_attr, W1, b1, W2, b2,
            U1, ub1, U2, ub2, gamma, beta):
    """Host prep + program build. Returns (nc, in_maps, meta)."""
    node_features = np.asarray(node_features, dtype=np.float32)
    edge_index = np.asarray(edge_index)
    edge_attr = np.asarray(edge_attr, dtype=np.float32)
    W1 = np.asarray(W1, dtype=np.float32); b1 = np.asarray(b1, dtype=np.float32)
    W2 = np.asarray(W2, dtype=np.float32); b2 = np.asarray(b2, dtype=np.float32)
    U1 = np.asarray(U1, dtype=np.float32); ub1 = np.asarray(ub1, dtype=np.float32)
    U2 = np.asarray(U2, dtype=np.float32)

    in_maps, meta = _prep(node_features, edge_index, edge_attr)
    gt = meta["gt"]
    has_b1 = bool(np.any(b1 != 0))
    has_b2 = bool(np.any(b2 != 0))

    w123 = np.stack([W1[:P], W1[P:2 * P], W1[2 * P:]], axis=1).astype(BF16)  # [128,3,128]
    shared = dict(
        W123=w123,
        W2=W2.astype(BF16),
        U1a=U1[:P].astype(BF16),
        U1b=U1[P:].astype(BF16),
        U2=U2.astype(BF16),
        ub1_t=ub1[:, None].astype(np.float32),
        iota_f=np.broadcast_to(np.arange(P, dtype=np.float32), (P, P)).copy(),
    )
    if has_b1:
        shared["ones_r"] = np.ones((1, P), dtype=np.float32).astype(BF16)
        shared["b1_r"] = b1[None, :].astype(BF16)
    if has_b2:
        shared["b2_1"] = b2[None, :].astype(BF16)
    for m in in_maps:
        if not has_b2:
            m.pop("deg_r", None)
        m.update(shared)
    nc = _build(gt, has_b1, has_b2)
    return nc, in_maps, meta


def finalize(res_results, meta, U1, ub1, U2, gamma, beta):
    """BatchNorm finalize + unshard on host."""
    slots_node = meta["slots_node"]
    updT = np.concatenate([r["updT_out"] for r in res_results], axis=1)
    stats = np.stack([r["stats_out"] for r in res_results]).astype(np.float64)

    ub1 = np.asarray(ub1, dtype=np.float64)
    U2_64 = np.asarray(U2, dtype=np.float64)
    gamma = np.asarray(gamma, dtype=np.float64)
    beta = np.asarray(beta, dtype=np.float64)

    # remove padding-slot contributions (identical constant column each)
    c_fake = np.log1p(np.exp(ub1)) @ U2_64
    n_fake = meta["n_fake"]
    tot_sum = stats[:, :, 0].sum(axis=0) - n_fake * c_fake
    tot_sq = stats[:, :, 1].sum(axis=0) - n_fake * c_fake ** 2
    mu = tot_sum / N
    var = tot_sq / N - mu ** 2
    scale = gamma / np.sqrt(var + EPS)
    shift = beta - mu * scale

    keep = slots_node >= 0
    out = np.empty((N, P), dtype=np.float32)
    out[slots_node[keep]] = (updT[:, keep].astype(np.float64) * scale[:, None]
                             + shift[:, None]).T.astype(np.float32)
    return out


def kernel(node_features, edge_index, edge_attr, W1, b1, W2, b2,
           U1, ub1, U2, ub2, gamma, beta):
    global LAST_RESULTS
    nc, in_maps, meta = prepare(node_features, edge_index, edge_attr, W1, b1,
                                W2, b2, U1, ub1, U2, ub2, gamma, beta)
    res = run_bass_kernel_spmd(nc, in_maps, core_ids=list(range(NCORES)), trace=TRACE)
    LAST_RESULTS = res
    return finalize(res.results, meta, U1, ub1, U2, gamma, beta)


